# revision 11
# baseline (speedup 1.0000x reference)
"""GATNet (3-layer single-head GAT, eval mode) on 8 Trainium2 NeuronCores.

Strategy (graph/data parallel, per sharding hint):
  - Nodes sharded contiguously across 8 cores (3750/core); every edge
    (incl. self-loops) is routed to the core owning its *destination*.
  - Per layer: each core computes h_ext_own = x_own @ [W | W@a_src | W@a_dst]
    for its own nodes; an AllGather replicates h_ext for all nodes.
  - Edges on each core are sorted by dst and packed into groups of <=1024
    edges covering <=128 consecutive dst rows (whole segments).  Per
    128-edge chunk: dma_gather source rows; build O_w[edge, dst_local] =
    is_equal(iota, dst_local) * softmax_weight on DVE; accumulate
    psum[dst, :] += O_w^T @ gathered_rows on the TensorEngine.  A constant
    1.0 column in each h_ext row makes the same matmul emit the softmax
    denominator.  Rows are normalized, biased, relu'd, PE-transposed and
    fed directly into the next layer's h_ext matmul.  Segment softmax
    uses no max subtraction (e in [-2, 6] for these weights; e is clamped
    at 80 before exp as insurance).
  - hd[dst] per edge comes from a per-core broadcast table (row r = hd of
    own node r replicated 64 wide) gathered with local dst ids; layer
    outputs are scattered to DRAM with batched dma_scatter_add (targets
    pre-zeroed; all indices within a batch are distinct).

HW notes (differ from bass_interp): dma_gather/dma_scatter_add index
tiles are a [16, n/16] int16 wrap REPLICATED on all 8 Q7 windows (128
partitions); indirect_dma_start services only one offset per partition.

The Bass program is identical on all 8 cores (SPMD); all data-dependent
routing lives in per-core index tensors computed here in numpy.
"""

import numpy as np
from contextlib import ExitStack

import concourse.bass as bass
import concourse.tile as tile
from concourse import bacc, mybir
from concourse.bass_utils import run_bass_kernel_spmd

F32 = mybir.dt.float32
F32R = mybir.dt.float32r
I16 = mybir.dt.int16
AF = mybir.ActivationFunctionType
ALU = mybir.AluOpType

N_CORES = 8
CPG = 8                    # chunks (of 128 edges) per group
EPG = 128 * CPG            # edges per group
SB = 8                     # groups per scatter-add batch

# h_ext row layout per layer: [h (fout) | hs | hd | 1.0 | zero pad] (elem floats)
LAYERS = [
    dict(fin=58, fout=300, hcols=302, elem=320),
    dict(fin=300, fout=100, hcols=102, elem=128),
    dict(fin=100, fout=1, hcols=3, elem=64),
]
HDW = 64                   # hd broadcast-table row width (256B min gather elem)
E_CLAMP = 80.0


def wrap_idx(a):
    """[..., n] int array -> [..., 128, n/16] int16 'wrapped + replicated'."""
    n = a.shape[-1]
    assert n % 16 == 0
    out = np.zeros(a.shape[:-1] + (16, n // 16), np.int16)
    i = np.arange(n)
    out[..., i % 16, i // 16] = a.astype(np.int16)
    return np.broadcast_to(
        out[..., None, :, :],
        a.shape[:-1] + (8, 16, n // 16)).reshape(a.shape[:-1] + (128, n // 16))


# ----------------------------------------------------------------- routing
def build_routing(src, dst, n_nodes, n_cores):
    shard = n_nodes // n_cores
    cores = []
    for c in range(n_cores):
        lo = c * shard
        m = (dst >= lo) & (dst < lo + shard)
        s_c = src[m]
        d_c = dst[m] - lo
        o = np.argsort(d_c, kind="stable")
        s_c, d_c = s_c[o].astype(np.int64), d_c[o].astype(np.int64)
        counts = np.bincount(d_c, minlength=shard)
        groups = []  # (base_dst, n_dst, edge_start, n_edges)
        base, cursor = 0, 0
        while base < shard:
            nd, ne = 0, 0
            while base + nd < shard and nd < 128:
                cnt = counts[base + nd]
                if ne + cnt > EPG:
                    break
                ne += cnt
                nd += 1
            assert nd > 0, "single dst exceeds EPG edges"
            groups.append((base, nd, cursor, ne))
            cursor += ne
            base += nd
        assert cursor == len(s_c)
        cores.append((s_c, d_c, groups))
    G = max(len(g) for _, _, g in cores)
    B = (G + SB - 1) // SB          # scatter batches

    metas = []
    for c in range(n_cores):
        s_c, d_c, groups = cores[c]
        src_e = np.zeros((G, EPG), np.int64)           # pad -> node 0
        dst_e = np.zeros((G, EPG), np.int64)           # pad -> local node 0
        dstloc = np.full((G, EPG), -1.0, np.float32)   # pad -> no one-hot match
        scat = np.zeros((G, 128), np.int64)
        for g in range(G):
            scat[g, :] = shard + g                     # pad -> per-group junk row
        for g, (base, nd, e0, ne) in enumerate(groups):
            src_e[g, :ne] = s_c[e0:e0 + ne]
            dst_e[g, :ne] = d_c[e0:e0 + ne]            # local ids
            dstloc[g, :ne] = (d_c[e0:e0 + ne] - base).astype(np.float32)
            scat[g, :nd] = base + np.arange(nd)

        # scatter batches: concat SB groups' scat columns -> 1024 ids
        scat_b = np.zeros((B, SB * 128), np.int64)
        for b in range(B):
            for k in range(SB):
                g = b * SB + k
                scat_b[b, k * 128:(k + 1) * 128] = (
                    scat[g] if g < G else shard + G + b)  # pad batch slot junk

        src16 = np.ascontiguousarray(
            wrap_idx(src_e).transpose(1, 0, 2).reshape(128, G * (EPG // 16)))
        dst16 = np.ascontiguousarray(
            wrap_idx(dst_e).transpose(1, 0, 2).reshape(128, G * (EPG // 16)))
        scat16 = np.ascontiguousarray(
            wrap_idx(scat_b).transpose(1, 0, 2).reshape(128, B * (SB * 128 // 16)))

        def pc_layout(a, dt):
            return np.ascontiguousarray(
                a.reshape(G, CPG, 128).transpose(2, 0, 1).reshape(128, G * CPG)
            ).astype(dt)

        metas.append(dict(
            src16=src16, dst16=dst16, scat16=scat16,
            dstloc=pc_layout(dstloc, np.float32)))
    return G, B, metas


# ------------------------------------------------------------- bass program
def build_program(n_nodes, G, B, use_f32r=True):
    shard = n_nodes // N_CORES
    nm = (shard + 127) // 128
    junk = G + B                    # junk rows appended to own tables
    L1, L2, L3 = LAYERS
    nc = bacc.Bacc("TRN2", target_bir_lowering=False, debug=False,
                   num_devices=N_CORES)

    def inp(name, shape, dtype):
        return nc.dram_tensor(name, list(shape), dtype, kind="ExternalInput")

    xT = inp("xT", (L1["fin"], shard), F32)
    W1e = inp("W1e", (L1["fin"], L1["hcols"]), F32)
    W2e = inp("W2e", (128, 3 * L2["hcols"]), F32)   # K-chunks [128,128,44]
    W3e = inp("W3e", (128, L3["hcols"]), F32)       # K-chunk [100] in rows 0:100
    b1b = inp("b1b", (128, L1["fout"]), F32)
    b2b = inp("b2b", (128, L2["fout"]), F32)
    b3b = inp("b3b", (128, 1), F32)
    iota = inp("iota", (128, 128), F32)
    ident = inp("ident", (128, 128), F32)
    src16 = inp("src16", (128, G * (EPG // 16)), I16)
    dst16 = inp("dst16", (128, G * (EPG // 16)), I16)
    scat16 = inp("scat16", (128, B * (SB * 128 // 16)), I16)
    dstloc = inp("dstloc", (128, G * CPG), F32)

    h1own = nc.dram_tensor("h1own", [shard, L1["elem"]], F32)
    hd1t = nc.dram_tensor("hd1t", [shard, HDW], F32)
    h1full = nc.dram_tensor("h1full", [n_nodes, L1["elem"]], F32,
                            addr_space="Shared")
    h2own = nc.dram_tensor("h2own", [shard + junk, L2["elem"]], F32)
    hd2t = nc.dram_tensor("hd2t", [shard + junk, HDW], F32)
    h2full = nc.dram_tensor("h2full", [n_nodes, L2["elem"]], F32,
                            addr_space="Shared")
    h3own = nc.dram_tensor("h3own", [shard + junk, L3["elem"]], F32)
    h3full = nc.dram_tensor("h3full", [n_nodes, L3["elem"]], F32,
                            addr_space="Shared")
    out_d = nc.dram_tensor("out", [shard + junk, HDW], F32,
                           kind="ExternalOutput")

    rg = [list(range(N_CORES))]

    def mmdt(ap):
        return ap.bitcast(F32R) if use_f32r else ap

    with tile.TileContext(nc) as tc, ExitStack() as ctx:
        cp = ctx.enter_context(tc.tile_pool(name="consts", bufs=1))
        wp = ctx.enter_context(tc.tile_pool(name="work", bufs=3))
        gp = ctx.enter_context(tc.tile_pool(name="gather", bufs=3))
        sp = ctx.enter_context(tc.tile_pool(name="scat", bufs=2))
        pp = ctx.enter_context(tc.tile_pool(name="ps_big", bufs=2, space="PSUM"))
        pt = ctx.enter_context(tc.tile_pool(name="ps_t", bufs=2, space="PSUM"))
        pn = ctx.enter_context(tc.tile_pool(name="ps_next", bufs=2, space="PSUM"))

        def load_const(t, shape, dtype):
            s = cp.tile(list(shape), dtype, tag=t.name)
            nc.sync.dma_start(out=s[:], in_=t.ap())
            return s

        xT_s = load_const(xT, (L1["fin"], shard), F32)
        W1_s = load_const(W1e, (L1["fin"], L1["hcols"]), F32)
        W2_s = load_const(W2e, (128, 3 * L2["hcols"]), F32)
        W3_s = load_const(W3e, (128, L3["hcols"]), F32)
        b1_s = load_const(b1b, (128, L1["fout"]), F32)
        b2_s = load_const(b2b, (128, L2["fout"]), F32)
        b3_s = load_const(b3b, (128, 1), F32)
        iota_s = load_const(iota, (128, 128), F32)
        id_s = load_const(ident, (128, 128), F32)
        src_s = load_const(src16, (128, G * (EPG // 16)), I16)
        dstg_s = load_const(dst16, (128, G * (EPG // 16)), I16)
        scat_s = load_const(scat16, (128, B * (SB * 128 // 16)), I16)
        dl_s = load_const(dstloc, (128, G * CPG), F32)

        # zero-fill scatter-add targets
        zt = cp.tile([128, 128], F32, tag="zeros")
        nc.vector.memset(zt[:], 0.0)
        for t, w in ((h2own, L2["elem"]), (hd2t, HDW), (h3own, L3["elem"]),
                     (out_d, HDW)):
            rows = t.shape[0]
            for r0 in range(0, rows, 128):
                rr = min(128, rows - r0)
                nc.sync.dma_start(out=t.ap()[r0:r0 + rr, :], in_=zt[0:rr, 0:w])

        # ---------------- phase A: h_ext1_own + hd1 table ------------------
        for m in range(nm):
            r0 = m * 128
            rows = min(128, shard - r0)
            ps = pp.tile([128, 512], F32, tag="psA")
            nc.tensor.matmul(ps[0:rows, 0:L1["hcols"]],
                             mmdt(xT_s[:, r0:r0 + rows]),
                             mmdt(W1_s[:, :]), start=True, stop=True)
            hx = wp.tile([128, L1["elem"]], F32, tag="hx1")
            nc.vector.tensor_copy(hx[0:rows, 0:L1["hcols"]],
                                  ps[0:rows, 0:L1["hcols"]])
            nc.vector.memset(hx[0:rows, L1["hcols"]:L1["hcols"] + 1], 1.0)
            nc.vector.memset(hx[0:rows, L1["hcols"] + 1:L1["elem"]], 0.0)
            nc.sync.dma_start(out=h1own.ap()[r0:r0 + rows, :], in_=hx[0:rows, :])
            hb = wp.tile([128, HDW], F32, tag="hb1")
            nc.vector.tensor_copy(
                hb[0:rows, :],
                ps[0:rows, L1["hcols"] - 1:L1["hcols"]].to_broadcast([rows, HDW]))
            nc.sync.dma_start(out=hd1t.ap()[r0:r0 + rows, :], in_=hb[0:rows, :])

        nc.gpsimd.collective_compute(
            "AllGather", ALU.bypass, replica_groups=rg,
            ins=[h1own.ap()], outs=[h1full.ap()])

        # ---------------- aggregation layer template ----------------------
        def agg_layer(li, hfull, hdtab, b_s, relu,
                      Wn_s=None, nk=None, next_hc=0, next_elem=0,
                      nxt_own=None, nxt_hdt=None, final=False, hd_col=0):
            elem, hc, fout = li["elem"], li["hcols"], li["fout"]
            n16 = EPG // 16
            sep_hdt = (not final) and (nxt_hdt is not nxt_own)
            hx_b = hd_b = ot_b = None
            for g in range(G):
                if g % SB == 0:
                    partial = g + SB > G
                    if not final:
                        hx_b = sp.tile([128, SB, next_elem], F32, tag="hxb")
                        if partial:
                            nc.vector.memset(
                                hx_b[:].rearrange("p a b -> p (a b)"), 0.0)
                        if sep_hdt:
                            hd_b = sp.tile([128, SB, HDW], F32, tag="hdb")
                            if partial:
                                nc.vector.memset(
                                    hd_b[:].rearrange("p a b -> p (a b)"), 0.0)
                    else:
                        ot_b = sp.tile([128, SB, HDW], F32, tag="otb")
                        if partial:
                            nc.vector.memset(
                                ot_b[:].rearrange("p a b -> p (a b)"), 0.0)
                gt = gp.tile([128, CPG, elem], F32, tag="gt")
                nc.gpsimd.dma_gather(
                    gt[:, :, :], hfull.ap(),
                    src_s[:, g * n16:(g + 1) * n16], EPG, EPG, elem)
                hdg = gp.tile([128, CPG, HDW], F32, tag="hdg")
                nc.gpsimd.dma_gather(
                    hdg[:, :, :], hdtab.ap(),
                    dstg_s[:, g * n16:(g + 1) * n16], EPG, EPG, HDW)
                # w = exp(min(lrelu(hs+hd), clamp))
                e_t = wp.tile([128, CPG], F32, tag="e")
                nc.vector.tensor_tensor(
                    out=e_t[:], in0=gt[:, :, hc - 2], in1=hdg[:, :, hd_col],
                    op=ALU.add)
                lr_t = wp.tile([128, CPG], F32, tag="lr")
                nc.vector.tensor_scalar_mul(lr_t[:], e_t[:], 0.2)
                nc.vector.tensor_tensor(
                    out=e_t[:], in0=e_t[:], in1=lr_t[:], op=ALU.max)
                w_t = wp.tile([128, CPG], F32, tag="w")
                nc.vector.tensor_scalar_min(w_t[:], e_t[:], E_CLAMP)
                nc.scalar.activation(w_t[:], w_t[:], AF.Exp)
                # accumulate over chunks
                ps = pp.tile([128, 512], F32, tag="psA")
                ow = wp.tile([128, CPG, 128], F32, tag="ow")
                for j in range(CPG):
                    nc.vector.tensor_scalar(
                        out=ow[:, j, :], in0=iota_s[:],
                        scalar1=dl_s[:, g * CPG + j:g * CPG + j + 1],
                        scalar2=w_t[:, j:j + 1],
                        op0=ALU.is_equal, op1=ALU.mult)
                    nc.tensor.matmul(ps[:, 0:hc + 1],
                                     mmdt(ow[:, j, :]),
                                     mmdt(gt[:, j, 0:hc + 1]),
                                     start=(j == 0), stop=(j == CPG - 1))
                # normalize rows, bias, relu
                s_t = wp.tile([128, 1], F32, tag="s")
                nc.vector.tensor_scalar_add(s_t[:], ps[:, hc:hc + 1], 1e-30)
                r_t = wp.tile([128, 1], F32, tag="r")
                nc.vector.reciprocal(r_t[:], s_t[:])
                ot = wp.tile([128, fout], F32, tag="ot")
                nc.vector.tensor_scalar(
                    out=ot[:], in0=ps[:, 0:fout], scalar1=r_t[:, 0:1],
                    scalar2=None, op0=ALU.mult)
                nc.vector.tensor_tensor(
                    out=ot[:], in0=ot[:], in1=b_s[:, 0:fout], op=ALU.add)
                if relu:
                    nc.scalar.activation(ot[:], ot[:], AF.Relu)
                k = g % SB
                last = (g == G - 1)
                if final:
                    nc.vector.tensor_copy(
                        ot_b[:, k, :], ot[:, 0:1].to_broadcast([128, HDW]))
                    if k == SB - 1 or last:
                        b_i = g // SB
                        nc.gpsimd.dma_scatter_add(
                            out_d.ap(), ot_b[:, :, :],
                            scat_s[:, b_i * (SB * 8):(b_i + 1) * (SB * 8)],
                            SB * 128, SB * 128, HDW)
                    continue
                # transpose fout in chunks of <=128 -> xT tiles
                xt = wp.tile([128, 128 * len(nk)], F32, tag="xt")
                for fc, kc in enumerate(nk):
                    c0 = fc * 128
                    pst = pt.tile([128, 128], F32, tag="pst")
                    nc.tensor.transpose(
                        out=pst[0:kc, :], in_=ot[:, c0:c0 + kc],
                        identity=id_s[:])
                    nc.vector.tensor_copy(xt[0:kc, c0:c0 + 128], pst[0:kc, :])
                # next-layer h_ext block
                ps2 = pn.tile([128, 128], F32, tag="ps2")
                for fc, kc in enumerate(nk):
                    nc.tensor.matmul(
                        ps2[:, 0:next_hc],
                        mmdt(xt[0:kc, fc * 128:fc * 128 + 128]),
                        mmdt(Wn_s[0:kc, fc * next_hc:(fc + 1) * next_hc]),
                        start=(fc == 0), stop=(fc == len(nk) - 1))
                nc.vector.tensor_copy(hx_b[:, k, 0:next_hc], ps2[:, 0:next_hc])
                nc.vector.memset(hx_b[:, k, next_hc:next_hc + 1], 1.0)
                if next_elem > next_hc + 1:
                    nc.vector.memset(hx_b[:, k, next_hc + 1:next_elem], 0.0)
                if sep_hdt:
                    nc.vector.tensor_copy(
                        hd_b[:, k, :],
                        ps2[:, next_hc - 1:next_hc].to_broadcast([128, HDW]))
                if k == SB - 1 or last:
                    b_i = g // SB
                    ssl = scat_s[:, b_i * (SB * 8):(b_i + 1) * (SB * 8)]
                    nc.gpsimd.dma_scatter_add(
                        nxt_own.ap(), hx_b[:, :, :], ssl,
                        SB * 128, SB * 128, next_elem)
                    if sep_hdt:
                        nc.gpsimd.dma_scatter_add(
                            nxt_hdt.ap(), hd_b[:, :, :], ssl,
                            SB * 128, SB * 128, HDW)

        # ---------------- L1 -> L2 -----------------------------------------
        agg_layer(L1, h1full, hd1t, b1_s, relu=True,
                  Wn_s=W2_s, nk=[128, 128, 44],
                  next_hc=L2["hcols"], next_elem=L2["elem"],
                  nxt_own=h2own, nxt_hdt=hd2t)
        nc.gpsimd.collective_compute(
            "AllGather", ALU.bypass, replica_groups=rg,
            ins=[h2own.ap()[0:shard, :]], outs=[h2full.ap()])

        # ---------------- L2 -> L3 -----------------------------------------
        agg_layer(L2, h2full, hd2t, b2_s, relu=True,
                  Wn_s=W3_s, nk=[100],
                  next_hc=L3["hcols"], next_elem=L3["elem"],
                  nxt_own=h3own, nxt_hdt=h3own)   # h3 row IS the hd3 table
        nc.gpsimd.collective_compute(
            "AllGather", ALU.bypass, replica_groups=rg,
            ins=[h3own.ap()[0:shard, :]], outs=[h3full.ap()])

        # ---------------- L3 (final) ---------------------------------------
        agg_layer(L3, h3full, h3own, b3_s, relu=False, final=True, hd_col=2)

    nc.compile()
    return nc


# ------------------------------------------------------------- host driver
def prepare(x, edge_index, Ws, as_, ads, bs):
    N = x.shape[0]
    loop = np.arange(N, dtype=np.int64)
    src = np.concatenate([np.asarray(edge_index[0], np.int64), loop])
    dst = np.concatenate([np.asarray(edge_index[1], np.int64), loop])
    G, B, metas = build_routing(src, dst, N, N_CORES)
    shard = N // N_CORES

    L1, L2, L3 = LAYERS
    W1e = np.concatenate(
        [Ws[0], Ws[0] @ as_[0][:, None], Ws[0] @ ads[0][:, None]],
        axis=1).astype(np.float32)
    W2raw = np.concatenate(
        [Ws[1], Ws[1] @ as_[1][:, None], Ws[1] @ ads[1][:, None]],
        axis=1).astype(np.float32)
    W3raw = np.concatenate(
        [Ws[2], Ws[2] @ as_[2][:, None], Ws[2] @ ads[2][:, None]],
        axis=1).astype(np.float32)
    hc2 = L2["hcols"]
    W2e = np.zeros((128, 3 * hc2), np.float32)
    for fc, kc in enumerate([128, 128, 44]):
        W2e[:kc, fc * hc2:(fc + 1) * hc2] = W2raw[fc * 128:fc * 128 + kc, :]
    W3e = np.zeros((128, L3["hcols"]), np.float32)
    W3e[:100, :] = W3raw

    xT_full = np.ascontiguousarray(np.asarray(x, np.float32).T)
    common = dict(
        W1e=W1e, W2e=W2e, W3e=W3e,
        b1b=np.broadcast_to(bs[0], (128, L1["fout"])).astype(np.float32).copy(),
        b2b=np.broadcast_to(bs[1], (128, L2["fout"])).astype(np.float32).copy(),
        b3b=np.broadcast_to(bs[2], (128, 1)).astype(np.float32).copy(),
        iota=np.broadcast_to(np.arange(128, dtype=np.float32),
                             (128, 128)).copy(),
        ident=np.eye(128, dtype=np.float32),
    )
    in_maps = []
    for c in range(N_CORES):
        m = metas[c]
        im = dict(common)
        im["xT"] = np.ascontiguousarray(xT_full[:, c * shard:(c + 1) * shard])
        for k in ("src16", "dst16", "scat16", "dstloc"):
            im[k] = m[k]
        in_maps.append(im)
    return G, B, in_maps, shard


_CACHE = {}


def kernel(x, edge_index, W1, a1s, a1d, b1, W2, a2s, a2d, b2, W3, a3s, a3d, b3,
           _trace=False, _use_f32r=True):
    x = np.asarray(x)
    G, B, in_maps, shard = prepare(
        x, np.asarray(edge_index),
        [np.asarray(W1), np.asarray(W2), np.asarray(W3)],
        [np.asarray(a1s), np.asarray(a2s), np.asarray(a3s)],
        [np.asarray(a1d), np.asarray(a2d), np.asarray(a3d)],
        [np.asarray(b1), np.asarray(b2), np.asarray(b3)])
    key = (x.shape[0], G, B, _use_f32r)
    if key not in _CACHE:
        _CACHE[key] = build_program(x.shape[0], G, B, use_f32r=_use_f32r)
    nc = _CACHE[key]
    res = run_bass_kernel_spmd(nc, in_maps, list(range(N_CORES)), trace=_trace)
    outs = [res.results[c]["out"][:shard, 0:1] for c in range(N_CORES)]
    full = np.concatenate(outs, axis=0).astype(np.float32)
    kernel._last = res
    return full


# revision 37
# speedup vs baseline: 1.0240x; 1.0240x over previous
"""GATNet (3-layer single-head GAT, eval mode) on 8 Trainium2 NeuronCores.

Strategy (graph/data parallel, per sharding hint):
  - Nodes sharded contiguously across 8 cores (3750/core); every edge
    (incl. self-loops) is routed to the core owning its *destination*.
  - Per layer: each core computes h_ext_own = x_own @ [W | W@a_src | W@a_dst]
    for its own nodes; an AllGather replicates h_ext for all nodes.
  - Edges on each core are sorted by dst and packed into groups of <=1024
    edges covering <=128 consecutive dst rows (whole segments).  Per
    128-edge chunk: dma_gather source rows; build O_w[edge, dst_local] =
    is_equal(iota, dst_local) * softmax_weight on DVE; accumulate
    psum[dst, :] += O_w^T @ gathered_rows on the TensorEngine.  A constant
    1.0 column in each h_ext row makes the same matmul emit the softmax
    denominator.  Rows are normalized, biased, relu'd, PE-transposed and
    fed directly into the next layer's h_ext matmul.  Segment softmax
    uses no max subtraction (e in [-2, 6] for these weights; e is clamped
    at 80 before exp as insurance).
  - hd[dst] per edge comes from a per-core broadcast table (row r = hd of
    own node r replicated 64 wide) gathered with local dst ids; layer
    outputs are scattered to DRAM with batched dma_scatter_add (targets
    pre-zeroed; all indices within a batch are distinct).

HW notes (differ from bass_interp): dma_gather/dma_scatter_add index
tiles are a [16, n/16] int16 wrap REPLICATED on all 8 Q7 windows (128
partitions); indirect_dma_start services only one offset per partition.

The Bass program is identical on all 8 cores (SPMD); all data-dependent
routing lives in per-core index tensors computed here in numpy.
"""

import numpy as np
from contextlib import ExitStack

import concourse.bass as bass
import concourse.tile as tile
from concourse import bacc, mybir
from concourse.bass_utils import run_bass_kernel_spmd

F32 = mybir.dt.float32
F32R = mybir.dt.float32r
I16 = mybir.dt.int16
AF = mybir.ActivationFunctionType
ALU = mybir.AluOpType

N_CORES = 8
CPG = 8                    # chunks (of 128 edges) per group
EPG = 128 * CPG            # edges per group
SB = 8                     # groups per scatter-add batch

# h_ext row layout per layer: [h (fout) | hs | hd | 1.0 | zero pad] (elem floats)
LAYERS = [
    dict(fin=58, fout=300, hcols=302, elem=320),
    dict(fin=300, fout=100, hcols=102, elem=128),
    dict(fin=100, fout=1, hcols=3, elem=64),
]
HDW = 64                   # hd broadcast-table row width (256B min gather elem)
E_CLAMP = 80.0


def wrap_idx(a):
    """[..., n] int array -> [..., 128, n/16] int16 'wrapped + replicated'."""
    n = a.shape[-1]
    assert n % 16 == 0
    out = np.zeros(a.shape[:-1] + (16, n // 16), np.int16)
    i = np.arange(n)
    out[..., i % 16, i // 16] = a.astype(np.int16)
    return np.broadcast_to(
        out[..., None, :, :],
        a.shape[:-1] + (8, 16, n // 16)).reshape(a.shape[:-1] + (128, n // 16))


# ----------------------------------------------------------------- routing
def build_routing(src, dst, n_nodes, n_cores):
    shard = n_nodes // n_cores
    cores = []
    for c in range(n_cores):
        lo = c * shard
        m = (dst >= lo) & (dst < lo + shard)
        s_c = src[m]
        d_c = dst[m] - lo
        o = np.argsort(d_c, kind="stable")
        s_c, d_c = s_c[o].astype(np.int64), d_c[o].astype(np.int64)
        counts = np.bincount(d_c, minlength=shard)
        groups = []  # (base_dst, n_dst, edge_start, n_edges)
        base, cursor = 0, 0
        while base < shard:
            nd, ne = 0, 0
            while base + nd < shard and nd < 128:
                cnt = counts[base + nd]
                if ne + cnt > EPG:
                    break
                ne += cnt
                nd += 1
            assert nd > 0, "single dst exceeds EPG edges"
            groups.append((base, nd, cursor, ne))
            cursor += ne
            base += nd
        assert cursor == len(s_c)
        cores.append((s_c, d_c, groups))
    G = max(len(g) for _, _, g in cores)
    B = (G + SB - 1) // SB          # scatter batches

    metas = []
    for c in range(n_cores):
        s_c, d_c, groups = cores[c]
        src_e = np.zeros((G, EPG), np.int64)           # pad -> node 0
        dst_e = np.zeros((G, EPG), np.int64)           # pad -> local node 0
        dstloc = np.full((G, EPG), -1.0, np.float32)   # pad -> no one-hot match
        scat = np.zeros((G, 128), np.int64)
        for g in range(G):
            scat[g, :] = shard + g                     # pad -> per-group junk row
        for g, (base, nd, e0, ne) in enumerate(groups):
            src_e[g, :ne] = s_c[e0:e0 + ne]
            dst_e[g, :ne] = d_c[e0:e0 + ne]            # local ids
            dstloc[g, :ne] = (d_c[e0:e0 + ne] - base).astype(np.float32)
            scat[g, :nd] = base + np.arange(nd)

        # scatter batches: concat SB groups' scat columns -> 1024 ids
        scat_b = np.zeros((B, SB * 128), np.int64)
        for b in range(B):
            for k in range(SB):
                g = b * SB + k
                scat_b[b, k * 128:(k + 1) * 128] = (
                    scat[g] if g < G else shard + G + b)  # pad batch slot junk

        src16 = np.ascontiguousarray(
            wrap_idx(src_e).transpose(1, 0, 2).reshape(128, G * (EPG // 16)))
        scatg16 = np.ascontiguousarray(
            wrap_idx(scat).transpose(1, 0, 2).reshape(128, G * 8))
        scat16 = np.ascontiguousarray(
            wrap_idx(scat_b).transpose(1, 0, 2).reshape(128, B * (SB * 128 // 16)))

        def pc_layout(a, dt):
            return np.ascontiguousarray(
                a.reshape(G, CPG, 128).transpose(2, 0, 1).reshape(128, G * CPG)
            ).astype(dt)

        metas.append(dict(
            src16=src16, scatg16=scatg16, scat16=scat16,
            dstloc=pc_layout(dstloc, np.float32)))
    return G, B, metas


# ------------------------------------------------------------- bass program
def build_program(n_nodes, G, B, use_f32r=True, single_core=False,
                  ablate=(), repeat=1):
    shard = n_nodes // N_CORES
    nm = (shard + 127) // 128
    junk = G + B                    # junk rows appended to own tables
    L1, L2, L3 = LAYERS
    nc = bacc.Bacc("TRN2", target_bir_lowering=False, debug=False,
                   num_devices=1 if single_core else N_CORES)

    def inp(name, shape, dtype):
        return nc.dram_tensor(name, list(shape), dtype, kind="ExternalInput")

    xT = inp("xT", (L1["fin"], shard), F32)
    W1e = inp("W1e", (L1["fin"], L1["hcols"]), F32)
    W2e = inp("W2e", (128, 3 * L2["hcols"]), F32)   # K-chunks [128,128,44]
    W3e = inp("W3e", (128, L3["hcols"]), F32)       # K-chunk [100] in rows 0:100
    b1b = inp("b1b", (128, L1["fout"]), F32)
    b2b = inp("b2b", (128, L2["fout"]), F32)
    b3b = inp("b3b", (128, 1), F32)
    iota = inp("iota", (128, 128), F32)
    ident = inp("ident", (128, 128), F32)
    src16 = inp("src16", (128, G * (EPG // 16)), I16)
    scatg16 = inp("scatg16", (128, G * 8), I16)
    scat16 = inp("scat16", (128, B * (SB * 128 // 16)), I16)
    dstloc = inp("dstloc", (128, G * CPG), F32)

    h1own = nc.dram_tensor("h1own", [shard, L1["elem"]], F32)
    hd1t = nc.dram_tensor("hd1t", [shard + junk, HDW], F32)
    h1full = nc.dram_tensor("h1full", [n_nodes, L1["elem"]], F32,
                            addr_space="Shared")
    h2own = nc.dram_tensor("h2own", [shard + junk, L2["elem"]], F32)
    hd2t = nc.dram_tensor("hd2t", [shard + junk, HDW], F32)
    h2full = nc.dram_tensor("h2full", [n_nodes, L2["elem"]], F32,
                            addr_space="Shared")
    h3own = nc.dram_tensor("h3own", [shard + junk, L3["elem"]], F32)
    h3full = nc.dram_tensor("h3full", [n_nodes, L3["elem"]], F32,
                            addr_space="Shared")
    out_d = nc.dram_tensor("out", [shard + junk, HDW], F32,
                           kind="ExternalOutput")

    rg = [list(range(N_CORES))]

    def mmdt(ap):
        return ap.bitcast(F32R) if use_f32r else ap

    with tile.TileContext(nc) as tc, ExitStack() as ctx:
        cp = ctx.enter_context(tc.tile_pool(name="consts", bufs=1))
        wp = ctx.enter_context(tc.tile_pool(name="work", bufs=3))
        gp = ctx.enter_context(tc.tile_pool(name="gather", bufs=3))
        sp = ctx.enter_context(tc.tile_pool(name="scat", bufs=2))
        pp = ctx.enter_context(tc.tile_pool(name="ps_big", bufs=2, space="PSUM"))
        pt = ctx.enter_context(tc.tile_pool(name="ps_t", bufs=2, space="PSUM"))
        pn = ctx.enter_context(tc.tile_pool(name="ps_next", bufs=2, space="PSUM"))
        pz = ctx.enter_context(tc.tile_pool(name="ps_z", bufs=2, space="PSUM"))

        def load_const(t, shape, dtype):
            s = cp.tile(list(shape), dtype, tag=t.name)
            nc.sync.dma_start(out=s[:], in_=t.ap())
            return s

        xT_s = load_const(xT, (L1["fin"], shard), F32)
        W1_s = load_const(W1e, (L1["fin"], L1["hcols"]), F32)
        W2_s = load_const(W2e, (128, 3 * L2["hcols"]), F32)
        W3_s = load_const(W3e, (128, L3["hcols"]), F32)
        b1_s = load_const(b1b, (128, L1["fout"]), F32)
        b2_s = load_const(b2b, (128, L2["fout"]), F32)
        b3_s = load_const(b3b, (128, 1), F32)
        iota_s = load_const(iota, (128, 128), F32)
        id_s = load_const(ident, (128, 128), F32)
        src_s = load_const(src16, (128, G * (EPG // 16)), I16)
        scatg_s = load_const(scatg16, (128, G * 8), I16)
        scat_s = load_const(scat16, (128, B * (SB * 128 // 16)), I16)
        dl_s = load_const(dstloc, (128, G * CPG), F32)

        # zero-fill scatter-add targets
        zt = cp.tile([128, 128], F32, tag="zeros")
        nc.vector.memset(zt[:], 0.0)
        ones1_s = cp.tile([1, 128], F32, tag="ones1")
        nc.vector.memset(ones1_s[:], 1.0)
        for t, w in ((h2own, L2["elem"]), (hd1t, HDW), (hd2t, HDW),
                     (h3own, L3["elem"]), (out_d, HDW)):
            rows = t.shape[0]
            for r0 in range(0, rows, 128):
                rr = min(128, rows - r0)
                nc.sync.dma_start(out=t.ap()[r0:r0 + rr, :], in_=zt[0:rr, 0:w])

        # ---------------- phase A: h_ext1_own + hd1 table ------------------
        def phase_a():
            for m in range(nm):
                r0 = m * 128
                rows = min(128, shard - r0)
                ps = pp.tile([128, 512], F32, tag="psA")
                nc.tensor.matmul(ps[0:rows, 0:L1["hcols"]],
                                 mmdt(xT_s[:, r0:r0 + rows]),
                                 mmdt(W1_s[:, :]), start=True, stop=True)
                hx = wp.tile([128, L1["elem"]], F32, tag="hx1")
                nc.vector.tensor_copy(hx[0:rows, 0:L1["hcols"]],
                                      ps[0:rows, 0:L1["hcols"]])
                nc.vector.memset(hx[0:rows, L1["hcols"]:L1["hcols"] + 1], 1.0)
                nc.vector.memset(hx[0:rows, L1["hcols"] + 1:L1["elem"]], 0.0)
                nc.sync.dma_start(out=h1own.ap()[r0:r0 + rows, :],
                                  in_=hx[0:rows, :])
                hb = wp.tile([128, HDW], F32, tag="hb1")
                nc.vector.tensor_copy(
                    hb[0:rows, :],
                    ps[0:rows, L1["hcols"] - 1:L1["hcols"]].to_broadcast(
                        [rows, HDW]))
                nc.sync.dma_start(out=hd1t.ap()[r0:r0 + rows, :],
                                  in_=hb[0:rows, :])

        def ag(own_ap, full_ap):
            if single_core:
                # timing-model stand-in: local write of the own shard
                nc.sync.dma_start(out=full_ap[0:shard, :], in_=own_ap)
            else:
                nc.gpsimd.collective_compute(
                    "AllGather", ALU.bypass, replica_groups=rg,
                    ins=[own_ap], outs=[full_ap])

        # ---------------- aggregation layer template ----------------------
        def agg_layer(li, hfull, hdtab, b_s, relu,
                      Wn_s=None, nk=None, next_hc=0, next_elem=0,
                      nxt_own=None, nxt_hdt=None, final=False, hd_col=0):
            elem, hc, fout = li["elem"], li["hcols"], li["fout"]
            n16 = EPG // 16
            sep_hdt = (not final) and (nxt_hdt is not nxt_own)
            hx_b = hd_b = ot_b = None
            for g in range(G):
                if g % SB == 0:
                    partial = g + SB > G
                    if not final:
                        hx_b = sp.tile([128, SB, next_elem], F32, tag="hxb")
                        if partial:
                            nc.vector.memset(
                                hx_b[:].rearrange("p a b -> p (a b)"), 0.0)
                        if sep_hdt:
                            hd_b = sp.tile([128, SB, HDW], F32, tag="hdb")
                            if partial:
                                nc.vector.memset(
                                    hd_b[:].rearrange("p a b -> p (a b)"), 0.0)
                    else:
                        ot_b = sp.tile([128, SB, HDW], F32, tag="otb")
                        if partial:
                            nc.vector.memset(
                                ot_b[:].rearrange("p a b -> p (a b)"), 0.0)
                gt = gp.tile([128, CPG, elem], F32, tag="gt")
                if "main_gather" in ablate:
                    nc.vector.memset(
                        gt[:].rearrange("p a b -> p (a b)"), 0.5)
                else:
                    nc.gpsimd.dma_gather(
                        gt[:, :, :], hfull.ap(),
                        src_s[:, g * n16:(g + 1) * n16], EPG, EPG, elem)
                # hd row for the group's <=128 dst slots
                hdb = gp.tile([128, 1, HDW], F32, tag="hdb2")
                if "hd_gather" in ablate:
                    nc.vector.memset(
                        hdb[:].rearrange("p a b -> p (a b)"), 0.5)
                else:
                    nc.gpsimd.dma_gather(
                        hdb[:, :, :], hdtab.ap(),
                        scatg_s[:, g * 8:(g + 1) * 8], 128, 128, HDW)
                # hd_row -> psum, then broadcast down partitions via ones-matmul
                pshd = pt.tile([128, 128], F32, tag="pst")
                nc.tensor.transpose(out=pshd[0:1, 0:128],
                                    in_=hdb[:, 0, hd_col:hd_col + 1],
                                    identity=id_s[:])
                hdr = wp.tile([1, 128], F32, tag="hdr")
                nc.vector.tensor_copy(hdr[:, :], pshd[0:1, 0:128])
                zps = pz.tile([128, 128], F32, tag="zps")
                nc.tensor.matmul(zps[:, :], mmdt(ones1_s[0:1, :]),
                                 mmdt(hdr[:, :]), start=True, stop=True)
                # accumulate over chunks
                ps = pp.tile([128, 512], F32, tag="psA")
                ow = wp.tile([128, CPG, 128], F32, tag="ow")
                for j in range(CPG):
                    if "onehot" not in ablate:
                        # wm = exp(lrelu(hd_bcast + hs_col))
                        wm = wp.tile([128, 128], F32, tag="wm")
                        zt2 = wp.tile([128, 128], F32, tag="zt2")
                        nc.vector.tensor_scalar(
                            out=zt2[:], in0=zps[:, :],
                            scalar1=gt[:, j, hc - 2:hc - 1], scalar2=None,
                            op0=ALU.add)
                        nc.vector.tensor_scalar_mul(wm[:], zt2[:], 0.2)
                        nc.vector.tensor_tensor(
                            out=wm[:], in0=wm[:], in1=zt2[:], op=ALU.max)
                        nc.scalar.activation(wm[:], wm[:], AF.Exp)
                        nc.vector.tensor_scalar(
                            out=ow[:, j, :], in0=iota_s[:],
                            scalar1=dl_s[:, g * CPG + j:g * CPG + j + 1],
                            scalar2=None, op0=ALU.is_equal)
                        nc.vector.tensor_tensor(
                            out=ow[:, j, :], in0=ow[:, j, :], in1=wm[:],
                            op=ALU.mult)
                    if "aggmm" not in ablate:
                        nc.tensor.matmul(ps[:, 0:hc + 1],
                                         mmdt(ow[:, j, :]),
                                         mmdt(gt[:, j, 0:hc + 1]),
                                         start=(j == 0), stop=(j == CPG - 1))
                # normalize rows, bias, relu
                s_t = wp.tile([128, 1], F32, tag="s")
                nc.vector.tensor_scalar_add(s_t[:], ps[:, hc:hc + 1], 1e-30)
                r_t = wp.tile([128, 1], F32, tag="r")
                nc.vector.reciprocal(r_t[:], s_t[:])
                ot = wp.tile([128, fout], F32, tag="ot")
                nc.vector.tensor_scalar(
                    out=ot[:], in0=ps[:, 0:fout], scalar1=r_t[:, 0:1],
                    scalar2=None, op0=ALU.mult)
                nc.vector.tensor_tensor(
                    out=ot[:], in0=ot[:], in1=b_s[:, 0:fout], op=ALU.add)
                if relu:
                    nc.scalar.activation(ot[:], ot[:], AF.Relu)
                k = g % SB
                last = (g == G - 1)
                if final:
                    nc.vector.tensor_copy(
                        ot_b[:, k, :], ot[:, 0:1].to_broadcast([128, HDW]))
                    if (k == SB - 1 or last) and "scatter" not in ablate:
                        b_i = g // SB
                        nc.gpsimd.dma_scatter_add(
                            out_d.ap(), ot_b[:, :, :],
                            scat_s[:, b_i * (SB * 8):(b_i + 1) * (SB * 8)],
                            SB * 128, SB * 128, HDW)
                    continue
                # transpose fout in chunks of <=128 -> xT tiles
                xt = wp.tile([128, 128 * len(nk)], F32, tag="xt")
                for fc, kc in enumerate(nk):
                    c0 = fc * 128
                    pst = pt.tile([128, 128], F32, tag="pst")
                    nc.tensor.transpose(
                        out=pst[0:kc, :], in_=ot[:, c0:c0 + kc],
                        identity=id_s[:])
                    nc.vector.tensor_copy(xt[0:kc, c0:c0 + 128], pst[0:kc, :])
                # next-layer h_ext block
                ps2 = pn.tile([128, 128], F32, tag="ps2")
                for fc, kc in enumerate(nk):
                    nc.tensor.matmul(
                        ps2[:, 0:next_hc],
                        mmdt(xt[0:kc, fc * 128:fc * 128 + 128]),
                        mmdt(Wn_s[0:kc, fc * next_hc:(fc + 1) * next_hc]),
                        start=(fc == 0), stop=(fc == len(nk) - 1))
                nc.vector.tensor_copy(hx_b[:, k, 0:next_hc], ps2[:, 0:next_hc])
                nc.vector.memset(hx_b[:, k, next_hc:next_hc + 1], 1.0)
                if next_elem > next_hc + 1:
                    nc.vector.memset(hx_b[:, k, next_hc + 1:next_elem], 0.0)
                if sep_hdt:
                    nc.vector.tensor_copy(
                        hd_b[:, k, :],
                        ps2[:, next_hc - 1:next_hc].to_broadcast([128, HDW]))
                if (k == SB - 1 or last) and "scatter" not in ablate:
                    b_i = g // SB
                    ssl = scat_s[:, b_i * (SB * 8):(b_i + 1) * (SB * 8)]
                    nc.gpsimd.dma_scatter_add(
                        nxt_own.ap(), hx_b[:, :, :], ssl,
                        SB * 128, SB * 128, next_elem)
                    if sep_hdt:
                        nc.gpsimd.dma_scatter_add(
                            nxt_hdt.ap(), hd_b[:, :, :], ssl,
                            SB * 128, SB * 128, HDW)

        for _rep in range(repeat):
            phase_a()
            ag(h1own.ap(), h1full.ap())
            # L1 -> L2
            agg_layer(L1, h1full, hd1t, b1_s, relu=True,
                      Wn_s=W2_s, nk=[128, 128, 44],
                      next_hc=L2["hcols"], next_elem=L2["elem"],
                      nxt_own=h2own, nxt_hdt=hd2t)
            ag(h2own.ap()[0:shard, :], h2full.ap())
            # L2 -> L3
            agg_layer(L2, h2full, hd2t, b2_s, relu=True,
                      Wn_s=W3_s, nk=[100],
                      next_hc=L3["hcols"], next_elem=L3["elem"],
                      nxt_own=h3own, nxt_hdt=h3own)   # h3 row IS the hd3 table
            ag(h3own.ap()[0:shard, :], h3full.ap())
            # L3 (final)
            agg_layer(L3, h3full, h3own, b3_s, relu=False, final=True,
                      hd_col=2)

    nc.compile()
    return nc


# ------------------------------------------------------------- host driver
def prepare(x, edge_index, Ws, as_, ads, bs):
    N = x.shape[0]
    loop = np.arange(N, dtype=np.int64)
    src = np.concatenate([np.asarray(edge_index[0], np.int64), loop])
    dst = np.concatenate([np.asarray(edge_index[1], np.int64), loop])
    G, B, metas = build_routing(src, dst, N, N_CORES)
    shard = N // N_CORES

    L1, L2, L3 = LAYERS
    W1e = np.concatenate(
        [Ws[0], Ws[0] @ as_[0][:, None], Ws[0] @ ads[0][:, None]],
        axis=1).astype(np.float32)
    W2raw = np.concatenate(
        [Ws[1], Ws[1] @ as_[1][:, None], Ws[1] @ ads[1][:, None]],
        axis=1).astype(np.float32)
    W3raw = np.concatenate(
        [Ws[2], Ws[2] @ as_[2][:, None], Ws[2] @ ads[2][:, None]],
        axis=1).astype(np.float32)
    hc2 = L2["hcols"]
    W2e = np.zeros((128, 3 * hc2), np.float32)
    for fc, kc in enumerate([128, 128, 44]):
        W2e[:kc, fc * hc2:(fc + 1) * hc2] = W2raw[fc * 128:fc * 128 + kc, :]
    W3e = np.zeros((128, L3["hcols"]), np.float32)
    W3e[:100, :] = W3raw

    xT_full = np.ascontiguousarray(np.asarray(x, np.float32).T)
    common = dict(
        W1e=W1e, W2e=W2e, W3e=W3e,
        b1b=np.broadcast_to(bs[0], (128, L1["fout"])).astype(np.float32).copy(),
        b2b=np.broadcast_to(bs[1], (128, L2["fout"])).astype(np.float32).copy(),
        b3b=np.broadcast_to(bs[2], (128, 1)).astype(np.float32).copy(),
        iota=np.broadcast_to(np.arange(128, dtype=np.float32),
                             (128, 128)).copy(),
        ident=np.eye(128, dtype=np.float32),
    )
    in_maps = []
    for c in range(N_CORES):
        m = metas[c]
        im = dict(common)
        im["xT"] = np.ascontiguousarray(xT_full[:, c * shard:(c + 1) * shard])
        for k in ("src16", "scatg16", "scat16", "dstloc"):
            im[k] = m[k]
        in_maps.append(im)
    return G, B, in_maps, shard


_CACHE = {}


def kernel(x, edge_index, W1, a1s, a1d, b1, W2, a2s, a2d, b2, W3, a3s, a3d, b3,
           _trace=False, _use_f32r=False):
    x = np.asarray(x)
    G, B, in_maps, shard = prepare(
        x, np.asarray(edge_index),
        [np.asarray(W1), np.asarray(W2), np.asarray(W3)],
        [np.asarray(a1s), np.asarray(a2s), np.asarray(a3s)],
        [np.asarray(a1d), np.asarray(a2d), np.asarray(a3d)],
        [np.asarray(b1), np.asarray(b2), np.asarray(b3)])
    key = (x.shape[0], G, B, _use_f32r)
    if key not in _CACHE:
        _CACHE[key] = build_program(x.shape[0], G, B, use_f32r=_use_f32r)
    nc = _CACHE[key]
    res = run_bass_kernel_spmd(nc, in_maps, list(range(N_CORES)), trace=_trace)
    outs = [res.results[c]["out"][:shard, 0:1] for c in range(N_CORES)]
    full = np.concatenate(outs, axis=0).astype(np.float32)
    kernel._last = res
    return full
